# revision 46
# baseline (speedup 1.0000x reference)
"""AttnBlock (GroupNorm -> single-head attention over 64x64 tokens -> proj -> residual)
for Trainium2, SPMD over 8 NeuronCores.  fp8e4 DoubleRow formulation.

Sharding: core = batch(4) x query-half(2) (token order is permutation-invariant
for GroupNorm stats and softmax attention; each core's query half is rotated to
the front of its token axis).

Algebraic structure (per core), with h = s (.) x + t the GroupNorm affine:
  S^T[j,i] = k_j . q_i   with k = Wk h (+ck), q = Wq h + cq
           = x_j^T z_i + g(i)               [g(i) const per query: softmax-invariant]
    where z = diag(s) (M0 (s (.) x_i)) + s (.) (M0 t + Wk^T bq),  M0 = Wk^T Wq
  et = exp(S/sqrt(C) - ln16)  (fp8; -ln16 keeps exp in e4m3 range)
  l = sum_j et (via 0.125-valued all-ones lhsT matmul -> lrb = 8/l broadcast)
  A[c,i] = sum_j x[c,j] et[j,i]   (attention applied to RAW x)
  a8 = A * lrb = 8 * (sum_j x p_ij)
  y = M1s^T a8 + bpp + x,  M1 = (Wp Wv)/8, M1s = diag(s) M1^T,
      bpp = bp + Wp Wv t + Wp bv   [v-bias and proj-bias deferred through linearity]

All heavy matmuls are fp8e4 MatmulPerfMode.DoubleRow (K=256/instr, 0.5 cyc/row).
GN stats come from host-staged xT8/xsqT8 via trivial ones-column matmuls.
"""

import math
import numpy as np
import ml_dtypes

import concourse.bass as bass
import concourse.mybir as mybir
import concourse.tile as tile

P = 128
C = 512
NCC = C // P          # 4 channel chunks
HW = 4096             # tokens per image
IHALF = 2048          # query tokens per core
NBLK = IHALF // 512   # 4 i-blocks
NJC = HW // P         # 32 j chunks of 128
GS = 16               # channels per group
EPS = 1e-6
ISC = 1.0 / math.sqrt(C)
LN16 = math.log(16.0)

F32 = mybir.dt.float32
BF16 = mybir.dt.bfloat16
FP8 = mybir.dt.float8e4
BF = ml_dtypes.bfloat16
E4 = ml_dtypes.float8_e4m3

DR = mybir.MatmulPerfMode.DoubleRow
ALU = mybir.AluOpType
AF = mybir.ActivationFunctionType


def _split_excess_waits(nc):
    """walrus accepts only ONE sync-wait per instruction; move extra waits
    onto same-engine NOPs placed immediately before."""
    for fn in nc.m.functions:
        for bb in fn.blocks:
            insts = list(bb.instructions)
            out = []
            changed = False
            for inst in insts:
                si = inst.sync_info
                if si is not None and len(si.on_wait) > 1:
                    waits = list(si.on_wait)
                    for k, w in enumerate(waits[:-1]):
                        nop = mybir.InstNoOp(
                            name=f"{inst.name}-ws{k}",
                            sync_info=mybir.SyncInfo(on_wait=[w], on_update=[]),
                            bass_nofuse=True,
                            engine=inst.engine,
                        )
                        out.append(nop)
                    inst.sync_info = mybir.SyncInfo(
                        on_wait=[waits[-1]], on_update=list(si.on_update)
                    )
                    changed = True
                out.append(inst)
            if changed:
                bb.instructions = out


def build_nc(split_waits=True):
    nc = bass.Bass()

    x8a_d = nc.declare_dram_parameter("x8a", [P, NCC, IHALF], FP8, isOutput=False)
    x8b_d = nc.declare_dram_parameter("x8b", [P, NCC, IHALF], FP8, isOutput=False)
    xt8_d = nc.declare_dram_parameter("xt8", [P, NJC, C], FP8, isOutput=False)
    xq8_d = nc.declare_dram_parameter("xq8", [P, NJC, C], FP8, isOutput=False)
    xres_d = nc.declare_dram_parameter("xres", [P, NCC, NBLK, 512], BF16, isOutput=False)
    m0t_d = nc.declare_dram_parameter("m0t", [P, NCC, C], BF16, isOutput=False)
    m1t_d = nc.declare_dram_parameter("m1t", [P, NCC, C], BF16, isOutput=False)
    gamma_d = nc.declare_dram_parameter("gamma_pc", [P, NCC], F32, isOutput=False)
    beta_d = nc.declare_dram_parameter("beta_pc", [P, NCC], F32, isOutput=False)
    wkbq_d = nc.declare_dram_parameter("wkbq_pc", [P, NCC], F32, isOutput=False)
    bpw_d = nc.declare_dram_parameter("bpw_pc", [P, NCC], F32, isOutput=False)
    ones8_d = nc.declare_dram_parameter("ones8", [P, 2, 1], FP8, isOutput=False)
    eighth8_d = nc.declare_dram_parameter("eighth8", [P, 2, P], FP8, isOutput=False)
    ind16_d = nc.declare_dram_parameter("ind16", [P, P // GS], F32, isOutput=False)
    bcast16_d = nc.declare_dram_parameter("bcast16", [P // GS, P], F32, isOutput=False)
    y_d = nc.declare_dram_parameter("yout", [P, NCC, IHALF], F32, isOutput=True)

    with tile.TileContext(nc) as tc:
        with (
            tc.tile_pool(name="big", bufs=1) as bpool,
            tc.tile_pool(name="const", bufs=1) as cpool,
            tc.tile_pool(name="gn", bufs=2) as gpool,
        ):
            x8a = bpool.tile([P, NCC, IHALF], FP8, tag="x8a")
            x8b = bpool.tile([P, NCC, IHALF], FP8, tag="x8b")

            def x8jc(jc):  # lhsT chunk for j-chunk jc: [P, NCC, 128]
                t = x8a if jc < 16 else x8b
                j0 = (jc % 16) * P
                return t, j0
            xt8 = bpool.tile([P, NJC, C], FP8, tag="xt8")
            xq8 = bpool.tile([P, NJC, C], FP8, tag="xq8")
            xres = bpool.tile([P, NCC, NBLK, 512], BF16, tag="xres")
            m0t = bpool.tile([P, NCC, C], BF16, tag="m0t")
            m1t = bpool.tile([P, NCC, C], BF16, tag="m1t")
            m0ts8 = bpool.tile([P, NCC, C], FP8, tag="m0ts8")
            m1ts8 = bpool.tile([P, NCC, C], FP8, tag="m1ts8")
            z8lo = bpool.tile([P, NCC, 2, 512], FP8, tag="z8lo")  # i-blocks 0,1 (DVE-evicted)
            z8hi = bpool.tile([P, NCC, 2, 512], FP8, tag="z8hi")  # i-blocks 2,3 (ACT-evicted)

            gamma_sb = cpool.tile([P, NCC], F32, tag="gamma")
            beta_sb = cpool.tile([P, NCC], F32, tag="beta")
            wkbq_sb = cpool.tile([P, NCC], F32, tag="wkbq")
            bpw_sb = cpool.tile([P, NCC], F32, tag="bpw")
            ones8_sb = cpool.tile([P, 2, 1], FP8, tag="ones8")
            eighth8_sb = cpool.tile([P, 2, P], FP8, tag="eighth8")
            ind16_sb = cpool.tile([P, P // GS], F32, tag="ind16")
            bcast16_sb = cpool.tile([P // GS, P], F32, tag="bcast16")
            eps_sb = cpool.tile([P // GS, 1], F32, tag="eps")
            negln16 = cpool.tile([P, 1], F32, tag="negln16")

            s_sb = gpool.tile([P, NCC], F32, tag="s")
            tbf = gpool.tile([P, NCC], BF16, tag="tbf")
            zadd_sb = gpool.tile([P, NCC], F32, tag="zadd")
            bpp_sb = gpool.tile([P, NCC], F32, tag="bpp")

            # ---- input DMAs: consts, stats operands (pair-interleaved), then the rest ----
            nc.vector.memset(eps_sb[:], EPS)
            nc.vector.memset(negln16[:], -LN16)
            # preload the sqrt ACT table at t~0 so the GN Sqrt pays no load
            sqd = gpool.tile([P // GS, 1], F32, tag="sqd")
            nc.scalar.activation(out=sqd[:], in_=eps_sb[:], func=AF.Sqrt,
                                 bias=eps_sb[:], scale=1.0)
            for t_sb, t_d in ((ones8_sb, ones8_d), (eighth8_sb, eighth8_d),
                              (ind16_sb, ind16_d), (bcast16_sb, bcast16_d),
                              (gamma_sb, gamma_d), (beta_sb, beta_d),
                              (wkbq_sb, wkbq_d), (bpw_sb, bpw_d)):
                nc.gpsimd.dma_start(out=t_sb[:], in_=t_d[:])
            # sync queue carries the critical-path order: m0t (small, unblocks the
            # s-folds) -> stats operands -> x8a (z conv) -> x8b -> m1t -> xres
            nc.sync.dma_start(out=m0t[:], in_=m0t_d[:])
            for i in range(4):
                nc.sync.dma_start(out=xt8[:, 8 * i:8 * i + 8, :], in_=xt8_d[:, 8 * i:8 * i + 8, :])
                nc.sync.dma_start(out=xq8[:, 8 * i:8 * i + 8, :], in_=xq8_d[:, 8 * i:8 * i + 8, :])
            for cc in (2, 3, 0, 1):  # qp pair (2,3) contracts first in the z conv
                nc.sync.dma_start(out=x8a[:, cc, :], in_=x8a_d[:, cc, :])
            for cc in range(NCC):
                nc.sync.dma_start(out=x8b[:, cc, :], in_=x8b_d[:, cc, :])
            nc.sync.dma_start(out=m1t[:], in_=m1t_d[:])
            nc.sync.dma_start(out=xres[:], in_=xres_d[:])

            # ---- GN stats: per-channel sum / sumsq via ones-column matmuls ----
            with tc.tile_pool(name="gps", bufs=1, space="PSUM") as gps:
                sum_ps = gps.tile([P, 512], F32, tag="sum")
                sum2_ps = gps.tile([P, 512], F32, tag="sum2")
                for cc in range(NCC):
                    for p in range(NJC // 2):
                        nc.tensor.matmul(
                            sum_ps[:, cc:cc + 1],
                            lhsT=xt8[:, 2 * p:2 * p + 2, cc * P:(cc + 1) * P],
                            rhs=ones8_sb[:],
                            start=(p == 0), stop=(p == NJC // 2 - 1), perf_mode=DR,
                        )
                    for p in range(NJC // 2):
                        nc.tensor.matmul(
                            sum2_ps[:, cc:cc + 1],
                            lhsT=xq8[:, 2 * p:2 * p + 2, cc * P:(cc + 1) * P],
                            rhs=ones8_sb[:],
                            start=(p == 0), stop=(p == NJC // 2 - 1), perf_mode=DR,
                        )

                # batched GN aggregation: one op per step over all 4 chunks
                mu_pc = gpool.tile([P, NCC], F32, tag="mupc")
                ex_pc = gpool.tile([P, NCC], F32, tag="expc")
                nc.vector.tensor_scalar_mul(mu_pc[:], sum_ps[:, 0:NCC], 1.0 / HW)
                nc.vector.tensor_scalar_mul(ex_pc[:], sum2_ps[:, 0:NCC], 1.0 / HW)
                gstat = gps.tile([P // GS, 2 * NCC], F32, tag="gstat")
                for cc in range(NCC):  # single-instruction groups: no region conflicts
                    nc.tensor.matmul(gstat[:, cc:cc + 1], lhsT=ind16_sb[:],
                                     rhs=mu_pc[:, cc:cc + 1], start=True, stop=True)
                    nc.tensor.matmul(gstat[:, NCC + cc:NCC + cc + 1], lhsT=ind16_sb[:],
                                     rhs=ex_pc[:, cc:cc + 1], start=True, stop=True)
                mg = gpool.tile([P // GS, 2 * NCC], F32, tag="mg")
                nc.vector.tensor_copy(out=mg[:, 0:NCC], in_=gstat[:, 0:NCC])
                musq = gpool.tile([P // GS, NCC], F32, tag="musq")
                nc.scalar.activation(out=musq[:], in_=gstat[:, 0:NCC], func=AF.Square)
                gvar = gpool.tile([P // GS, NCC], F32, tag="gvar")
                nc.vector.tensor_tensor(gvar[:], gstat[:, NCC:2 * NCC], musq[:], ALU.subtract)
                nc.scalar.activation(out=mg[:, NCC:2 * NCC], in_=gvar[:],
                                     func=AF.Sqrt, bias=eps_sb[:], scale=1.0)
                nc.vector.reciprocal(out=mg[:, NCC:2 * NCC], in_=mg[:, NCC:2 * NCC])
                bps = gps.tile([P, 2 * NCC], F32, tag="bc")
                nc.tensor.matmul(bps[:], lhsT=bcast16_sb[:], rhs=mg[:], start=True, stop=True)
                nc.vector.tensor_tensor(s_sb[:], bps[:, NCC:2 * NCC], gamma_sb[:], ALU.mult)
                tf = gpool.tile([P, NCC], F32, tag="tf")
                nc.vector.tensor_tensor(tf[:], bps[:, 0:NCC], s_sb[:], ALU.mult)
                nc.vector.tensor_tensor(tf[:], beta_sb[:], tf[:], ALU.subtract)
                nc.vector.tensor_copy(out=tbf[:], in_=tf[:])

                # ---- fold s into M0^T -> fp8 first (gates z matmuls): DVE || ACT ----
                for cc in range(NCC):
                    if cc % 2 == 0:
                        nc.vector.tensor_scalar(
                            out=m0ts8[:, cc, :], in0=m0t[:, cc, :],
                            scalar1=s_sb[:, cc:cc + 1], scalar2=None, op0=ALU.mult,
                        )
                    else:
                        nc.scalar.activation(
                            out=m0ts8[:, cc, :], in_=m0t[:, cc, :],
                            func=AF.Copy, scale=s_sb[:, cc:cc + 1],
                        )
                # preload exp table; input dep on m0ts8 forces it AFTER the Copy
                # folds in the scheduled ACT order (output is never read)
                dummy = gpool.tile([P, 1], F32, tag="dummy")
                nc.scalar.activation(out=dummy[:], in_=m0ts8[:, 3, 0:1],
                                     func=AF.Exp, bias=negln16[:], scale=0.0)

                # ---- bias fold: zadd = s*(M0 t + wkbq) (gates only z evicts) ----
                zadd_ps = gps.tile([P, 512], F32, tag="zaddp")
                for oc in range(NCC):
                    for cc in range(NCC):
                        nc.tensor.matmul(
                            zadd_ps[:, oc:oc + 1],
                            lhsT=m0t[:, cc, oc * P:(oc + 1) * P], rhs=tbf[:, cc:cc + 1],
                            start=(cc == 0), stop=(cc == NCC - 1),
                        )
                    nc.vector.tensor_scalar(
                        out=zadd_sb[:, oc:oc + 1], in0=zadd_ps[:, oc:oc + 1],
                        scalar1=wkbq_sb[:, oc:oc + 1], scalar2=s_sb[:, oc:oc + 1],
                        op0=ALU.add, op1=ALU.mult,
                    )

            # ---- z conv: z = s*(M0 (s.x_ihalf)) + zadd ----
            # separate psum tiles per evicting engine: a tile's readers are
            # chained by the framework, so sharing one wide tile would
            # serialize the DVE and ACT evicts
            with (
                tc.tile_pool(name="zpa", bufs=2, space="PSUM") as zpa,
                tc.tile_pool(name="zpb", bufs=2, space="PSUM") as zpb,
            ):
                for oc in range(NCC):
                    wa = zpa.tile([P, 2, 512], F32, tag="zwa")
                    wb = zpb.tile([P, 2, 512], F32, tag="zwb")
                    for it in range(NBLK):
                        w = wa if it < 2 else wb
                        for gi, qp in enumerate((1, 0)):  # pair (2,3) first: its x8a lands first
                            nc.tensor.matmul(
                                w[:, it % 2, :],
                                lhsT=m0ts8[:, 2 * qp:2 * qp + 2, oc * P:(oc + 1) * P],
                                rhs=x8a[:, 2 * qp:2 * qp + 2, it * 512:(it + 1) * 512],
                                start=(gi == 0), stop=(gi == 1), perf_mode=DR,
                            )
                    nc.vector.tensor_scalar(
                        out=z8lo[:, oc, :, :], in0=wa[:],
                        scalar1=s_sb[:, oc:oc + 1], scalar2=zadd_sb[:, oc:oc + 1],
                        op0=ALU.mult, op1=ALU.add,
                    )
                    nc.scalar.activation(
                        out=z8hi[:, oc, :, :], in_=wb[:],
                        func=AF.Identity, bias=zadd_sb[:, oc:oc + 1],
                        scale=s_sb[:, oc:oc + 1],
                    )
                # deferred: fold s into M1^T on Pool (needed first at y(ib0))
                for cc in range(NCC):
                    nc.gpsimd.tensor_scalar(
                        out=m1ts8[:, cc, :], in0=m1t[:, cc, :],
                        scalar1=s_sb[:, cc:cc + 1], scalar2=None, op0=ALU.mult,
                    )

            # ---- attention (software-pipelined across i-blocks) ----
            with (
                tc.tile_pool(name="st", bufs=2, space="PSUM") as stpool,
                tc.tile_pool(name="a0", bufs=1, space="PSUM") as a0pool,
                tc.tile_pool(name="a1", bufs=1, space="PSUM") as a1pool,
                tc.tile_pool(name="lp", bufs=1, space="PSUM") as lpool,
                tc.tile_pool(name="yp", bufs=1, space="PSUM") as ypool,
                tc.tile_pool(name="et", bufs=2) as etpool,
                tc.tile_pool(name="lrb", bufs=2) as lrbpool,
                tc.tile_pool(name="a8", bufs=2) as a8pool,
                tc.tile_pool(name="ost", bufs=4) as ostpool,
            ):
                post_q = []
                av_tiles = {}

                def drain(n):
                    for _ in range(n):
                        if post_q:
                            post_q.pop(0)()

                # deferred bpp = bp + wpbv + 8*(M1 t): tiny matmuls on the y bank
                bpp_ps = ypool.tile([P, 512], F32, tag="y", name="bpp_ps")
                for oc in range(NCC):
                    for cc in range(NCC):
                        nc.tensor.matmul(
                            bpp_ps[:, oc:oc + 1],
                            lhsT=m1t[:, cc, oc * P:(oc + 1) * P], rhs=tbf[:, cc:cc + 1],
                            start=(cc == 0), stop=(cc == NCC - 1),
                        )
                    nc.vector.tensor_scalar(
                        out=bpp_sb[:, oc:oc + 1], in0=bpp_ps[:, oc:oc + 1],
                        scalar1=8.0, scalar2=bpw_sb[:, oc:oc + 1],
                        op0=ALU.mult, op1=ALU.add,
                    )

                # process the z8hi i-blocks first: the ACT z-evict chain finishes
                # before the DVE one, so the race can start earlier
                for pos, ib in enumerate((2, 3, 0, 1)):
                    first, last = pos == 0, pos == NBLK - 1
                    isl = slice(ib * 512, (ib + 1) * 512)
                    et = etpool.tile([P, NJC, 512], FP8, tag="et", name=f"et{ib}")
                    l_ps = lpool.tile([P, 512], F32, tag="l")
                    a0 = a0pool.tile([P, 512], F32, tag="a0")
                    lrb = lrbpool.tile([P, 512], F32, tag="lrb", name=f"lrb{ib}")
                    a8t = a8pool.tile([P, NCC, 512], FP8, tag="a8", name=f"a8_{ib}")

                    for g in range(16):
                        st = stpool.tile([P, 2, 512], F32, tag="st")
                        zsrc = z8lo if ib < 2 else z8hi
                        for jl in range(2):
                            jc = 2 * g + jl
                            xsrc, j0 = x8jc(jc)
                            for qp in range(2):
                                nc.tensor.matmul(
                                    st[:, jl, :],
                                    lhsT=xsrc[:, 2 * qp:2 * qp + 2, j0:j0 + P],
                                    rhs=zsrc[:, 2 * qp:2 * qp + 2, ib % 2, :],
                                    start=(qp == 0), stop=(qp == 1), perf_mode=DR,
                                )
                        nc.scalar.activation(
                            out=et[:, 2 * g:2 * g + 2, :], in_=st[:],
                            func=AF.Exp, bias=negln16[:], scale=ISC,
                        )
                        drain(1)
                        nc.tensor.matmul(
                            l_ps[:], lhsT=eighth8_sb[:], rhs=et[:, 2 * g:2 * g + 2, :],
                            start=(g == 0), stop=(g == 15), perf_mode=DR,
                        )
                        nc.tensor.matmul(
                            a0[:], lhsT=xt8[:, 2 * g:2 * g + 2, 0:P],
                            rhs=et[:, 2 * g:2 * g + 2, :],
                            start=(g == 0), stop=(g == 15), perf_mode=DR,
                        )
                        if first:
                            # A1 is free during the first race: trail cc1 in-race too
                            a1f = av_tiles.get((ib, 1))
                            if a1f is None:
                                a1f = a1pool.tile([P, 512], F32, tag="a1", name="a1_f_1")
                                av_tiles[(ib, 1)] = a1f
                            nc.tensor.matmul(
                                a1f[:], lhsT=xt8[:, 2 * g:2 * g + 2, P:2 * P],
                                rhs=et[:, 2 * g:2 * g + 2, :],
                                start=(g == 0), stop=(g == 15), perf_mode=DR,
                            )

                    nc.vector.reciprocal(out=lrb[:], in_=l_ps[:])
                    nc.vector.tensor_tensor(a8t[:, 0, :], a0[:], lrb[:], ALU.mult)
                    if first:
                        nc.vector.tensor_tensor(
                            a8t[:, 1, :], av_tiles[(ib, 1)][:], lrb[:], ALU.mult)

                    # post work for this ib: AV cc1..3 on the A1 bank, then y
                    def mk_av(ib_, cc, et_, a8t_, lrb_, prange, last_):
                        def run(ib_=ib_, cc=cc, et_=et_, a8t_=a8t_, lrb_=lrb_,
                                prange=prange, last_=last_):
                            a1 = av_tiles.get((ib_, cc))
                            if a1 is None:
                                if last_ and cc == 2:
                                    a1 = a0pool.tile([P, 512], F32, tag="a0",
                                                     name=f"a1_{ib_}_{cc}")
                                elif last_ and cc == 3:
                                    # tail: the l bank is retired after its recip
                                    a1 = lpool.tile([P, 512], F32, tag="l",
                                                    name=f"a1_{ib_}_{cc}")
                                else:
                                    a1 = a1pool.tile([P, 512], F32, tag="a1",
                                                     name=f"a1_{ib_}_{cc}")
                                av_tiles[(ib_, cc)] = a1
                            for p_ in prange:
                                nc.tensor.matmul(
                                    a1[:], lhsT=xt8[:, 2 * p_:2 * p_ + 2, cc * P:(cc + 1) * P],
                                    rhs=et_[:, 2 * p_:2 * p_ + 2, :],
                                    start=(p_ == 0), stop=(p_ == 15), perf_mode=DR,
                                )
                            if prange[-1] == 15:
                                nc.vector.tensor_tensor(
                                    a8t_[:, cc, :], a1[:], lrb_[:], ALU.mult)
                        return run

                    def mk_y(ib_, oc, a8t_, isl_, last_):
                        def run(ib_=ib_, oc=oc, a8t_=a8t_, isl_=isl_, last_=last_):
                            if last_ and oc % 2 == 1:  # keep bank alternation
                                yp = lpool.tile([P, 512], F32, tag="l",
                                                name=f"y{ib_}_{oc}")
                            else:
                                yp = ypool.tile([P, 512], F32, tag="y", name=f"y{ib_}_{oc}")
                            for qp in range(2):
                                nc.tensor.matmul(
                                    yp[:],
                                    lhsT=m1ts8[:, 2 * qp:2 * qp + 2, oc * P:(oc + 1) * P],
                                    rhs=a8t_[:, 2 * qp:2 * qp + 2, :],
                                    start=(qp == 0), stop=(qp == 1), perf_mode=DR,
                                )
                            ost = ostpool.tile([P, 512], F32, tag="ost")
                            if last_ and oc == 1:
                                # tail: keep DVE free — bias-add on ACT, residual on Pool
                                ost1 = ostpool.tile([P, 512], F32, tag="ost1")
                                nc.scalar.activation(
                                    out=ost1[:], in_=yp[:], func=AF.Identity,
                                    bias=bpp_sb[:, oc:oc + 1], scale=1.0,
                                )
                                nc.gpsimd.tensor_tensor(
                                    ost[:], ost1[:], xres[:, oc, ib_, :], ALU.add)
                            else:
                                nc.vector.scalar_tensor_tensor(
                                    out=ost[:], in0=yp[:], scalar=bpp_sb[:, oc:oc + 1],
                                    in1=xres[:, oc, ib_, :], op0=ALU.add, op1=ALU.add,
                                )
                            if last_:
                                nc.sync.dma_start(out=y_d[:, oc, isl_], in_=ost[:])
                            else:
                                nc.gpsimd.dma_start(out=y_d[:, oc, isl_], in_=ost[:])
                        return run

                    prs = ([0, 1, 2, 3], [4, 5, 6, 7], [8, 9, 10, 11], [12, 13, 14, 15])
                    if last:
                        # tail: interleave cc1 (A1) with cc2 (A0) so they run in parallel
                        for pr1, pr2 in zip(prs, prs):
                            post_q.append(mk_av(ib, 1, et, a8t, lrb, pr1, last))
                            post_q.append(mk_av(ib, 2, et, a8t, lrb, pr2, last))
                        for pr in prs:
                            post_q.append(mk_av(ib, 3, et, a8t, lrb, pr, last))
                    else:
                        for cc in ((2, 3) if first else (1, 2, 3)):
                            for pr in prs:
                                post_q.append(mk_av(ib, cc, et, a8t, lrb, pr, last))
                    for oc in range(NCC):
                        post_q.append(mk_y(ib, oc, a8t, isl, last))

                drain(len(post_q))

    if split_waits:
        _split_excess_waits(nc)
    return nc


_NC = None


def _get_nc():
    global _NC
    if _NC is None:
        _NC = build_nc()
    return _NC


def _build_in_maps(x, gamma, beta, Wq, bq, Wk, bk, Wv, bv, Wp, bp):
    x = np.asarray(x, dtype=np.float32)
    B, c, H, W = x.shape
    assert (B, c, H, W) == (4, C, 64, 64)

    def pc(v):  # [C] -> [P, NCC]
        return np.ascontiguousarray(np.asarray(v, np.float32).reshape(NCC, P).T)

    Wqf = np.asarray(Wq, np.float64)
    Wkf = np.asarray(Wk, np.float64)
    Wvf = np.asarray(Wv, np.float64)
    Wpf = np.asarray(Wp, np.float64)
    M0 = (Wkf.T @ Wqf).astype(np.float32)          # [o, c]
    M1 = ((Wpf @ Wvf) / 8.0).astype(np.float32)    # [o, c]

    def chunk_t(M):  # [o, c] -> lhsT layout [P, NCC, C]: [p, cc, o] = M[o, cc*128+p]
        return np.ascontiguousarray(M.T.reshape(NCC, P, C).transpose(1, 0, 2))

    ind16 = np.zeros((P, P // GS), np.float32)
    ind16[np.arange(P), np.arange(P) // GS] = 1.0 / GS
    bcast16 = np.zeros((P // GS, P), np.float32)
    bcast16[np.arange(P) // GS, np.arange(P)] = 1.0

    shared = {
        "m0t": chunk_t(M0).astype(BF),
        "m1t": chunk_t(M1).astype(BF),
        "gamma_pc": pc(gamma), "beta_pc": pc(beta),
        "wkbq_pc": pc(Wkf.T @ np.asarray(bq, np.float64)),
        "bpw_pc": pc(np.asarray(bp, np.float64) + Wpf @ np.asarray(bv, np.float64)),
        "ones8": np.ones((P, 2, 1), np.float32).astype(E4),
        "eighth8": np.full((P, 2, P), 0.125, np.float32).astype(E4),
        "ind16": ind16, "bcast16": bcast16,
    }

    xf = x.reshape(B, C, HW)
    in_maps = []
    for core in range(8):
        b, half = divmod(core, 2)
        xb = xf[b]
        if half == 0:
            x_bc = xb
        else:
            x_bc = np.concatenate([xb[:, IHALF:], xb[:, :IHALF]], axis=1)
        x_bc = np.ascontiguousarray(x_bc)
        x8 = x_bc.reshape(NCC, P, HW).transpose(1, 0, 2).astype(E4)
        x8a = np.ascontiguousarray(x8[:, :, :IHALF])
        x8b = np.ascontiguousarray(x8[:, :, IHALF:])
        xT = np.ascontiguousarray(
            x_bc.T.reshape(NJC, P, C).transpose(1, 0, 2))
        xt8 = xT.astype(E4)
        xq8 = (xT.astype(np.float32) ** 2).astype(E4)
        xres = np.ascontiguousarray(
            x_bc[:, :IHALF].reshape(NCC, P, NBLK, 512).transpose(1, 0, 2, 3)).astype(BF)
        in_maps.append({
            "x8a": x8a, "x8b": x8b, "xt8": xt8, "xq8": xq8, "xres": xres, **shared,
        })
    return in_maps


def _core0_feed(inputs):
    """Input map for core 0 (batch 0, first query half) — used by test harnesses."""
    return _build_in_maps(**inputs)[0]


def kernel(x, gamma, beta, Wq, bq, Wk, bk, Wv, bv, Wp, bp):
    nc = _get_nc()
    in_maps = _build_in_maps(x, gamma, beta, Wq, bq, Wk, bk, Wv, bv, Wp, bp)

    from concourse.bass_utils import run_bass_kernel_spmd

    res = run_bass_kernel_spmd(nc, in_maps, list(range(8)))

    B = 4
    out = np.empty((B, C, HW), np.float32)
    for core in range(8):
        b, half = divmod(core, 2)
        y = res.results[core]["yout"]  # [P, NCC, IHALF]
        out[b, :, half * IHALF:(half + 1) * IHALF] = (
            y.transpose(1, 0, 2).reshape(C, IHALF))
    return out.reshape(B, C, 64, 64)
